# revision 29
# baseline (speedup 1.0000x reference)
"""Exaone GQA flash-attention block on 8 Trainium2 NeuronCores.

Sharding: each pair of cores (2p, 2p+1) handles prefill sequence p (S=1024).
Within a pair, q-tokens are split by 256-blocks {0,3} / {1,2} so causal attention
work balances; K/V are computed per-core for the context each core needs
(zero-padded to 1024). No cross-core communication: every core produces the
final output rows for its own 512 q-tokens; the host concatenates.

Device algorithm (all matmuls bf16, fp32 accumulation):
  hs^T via PE transposes that chase the fp32 HWDGE loads tile-by-tile (keeps
       the tensor engine warm from the first microseconds); the bf16 cast
       rides the PSUM->SBUF copy
  weights are cast fp32->bf16 inside gpsimd-initiated DMAs (SWDGE casts),
       so no compute engine spends cycles casting
  qT = Wq^T @ hsT in a [128, 16 head-slots, 512] layout (row halves = even/odd
       kv-head parity so scores matmuls row-pack the PE array), NeoX rope via a
       +-1 rotation matmul and two multiply-adds
  kT likewise [128, 4 kv-pairs, 1024]; V natural [tok, ch] with an appended
       ones column so the PV matmul also produces the softmax denominator
  scoresT = kT^T @ qT per (kv, 128-q-chunk, key-block), exp on ACT with a
       per-partition additive bias (kills invisible blocks), multiplicative
       triangle masks only on the two possible diagonal positions per chunk
  attn^T accumulated in PSUM [65, 4 heads, 128], normalized by the broadcast
       reciprocal of the ones-row, written straight into the out-proj lhsT
       layout; out = attn^T.T @ Wo streamed per 256-wide output chunk.
"""
import sys
sys.path.insert(0, '/opt/trn_rl_repo')

from contextlib import ExitStack

import ml_dtypes
import numpy as np

import concourse.bass as bass
import concourse.mybir as mybir
import concourse.tile as tile
from concourse import bacc
from concourse.bass_utils import run_bass_kernel_spmd
from concourse.masks import make_identity

F32 = mybir.dt.float32
BF16 = mybir.dt.bfloat16
AF = mybir.ActivationFunctionType
MUL = mybir.AluOpType.mult
ADD = mybir.AluOpType.add

B, S, D = 4, 1024, 2048
HQ, HKV, HD = 32, 8, 64
SCALE = HD ** -0.5
NQ = 512                      # q tokens per core
CSLOT2 = (4, 8)               # key-blocks processed per 256-q-chunk (uniform)
MASK_POS2 = ((0, 1, 2, 3), (4, 5, 6, 7))  # masked kb positions per 256-chunk
NEG = -1e30


def build_nc():
    nc = bacc.Bacc("TRN2", target_bir_lowering=False, debug=False,
                   num_devices=8, num_swdge_queues=4)

    hs_ctx = nc.dram_tensor("hs_ctx", [S, D], F32, kind="ExternalInput")
    hs_q = nc.dram_tensor("hs_q", [NQ, D], F32, kind="ExternalInput")
    cos_ctx = nc.dram_tensor("cos_ctx", [S, 32], F32, kind="ExternalInput")
    sin_ctx = nc.dram_tensor("sin_ctx", [S, 32], F32, kind="ExternalInput")
    cos_q = nc.dram_tensor("cos_q", [NQ, 32], F32, kind="ExternalInput")
    sin_q = nc.dram_tensor("sin_q", [NQ, 32], F32, kind="ExternalInput")
    wq = nc.dram_tensor("wq", [D, HQ * HD], F32, kind="ExternalInput")
    wk = nc.dram_tensor("wk", [D, HKV * HD], F32, kind="ExternalInput")
    wv = nc.dram_tensor("wv", [D, HKV * HD], F32, kind="ExternalInput")
    wo = nc.dram_tensor("wo", [HQ * HD, D], F32, kind="ExternalInput")
    rot_in = nc.dram_tensor("rot", [128, 128], BF16, kind="ExternalInput")
    masks_in = nc.dram_tensor("masks", [128, 2, 4, 256], BF16, kind="ExternalInput")
    ident_in = nc.dram_tensor("ident", [128, 128], F32, kind="ExternalInput")
    out = nc.dram_tensor("out", [NQ, D], F32, kind="ExternalOutput")

    wk_r = wk.rearrange("(ko ki) n -> ki ko n", ki=128)
    wv_r = wv.rearrange("(ko ki) n -> ki ko n", ki=128)
    wo_r = wo.rearrange("(ko ki) n -> ki ko n", ki=128)
    # Wq columns: col = 512a + 256r + 64i + c  (a: slot group, r: kv parity,
    # i: slot-in-group, c: head dim). Per group a the 512 cols are contiguous.
    wq_r = wq.rearrange("(ko ki) (a n) -> ki ko a n", ki=128, a=4)
    hsc_r = hs_ctx.rearrange("(o p) d -> p o d", p=128)
    hsq_r = hs_q.rearrange("(o p) d -> p o d", p=128)

    with tile.TileContext(nc) as tc:
        with ExitStack() as ctx:
            pool = lambda *a, **k: ctx.enter_context(tc.tile_pool(*a, **k))
            qT_p = pool(name="qT", bufs=1)
            kT_p = pool(name="kT", bufs=1)
            v_p = pool(name="vsb", bufs=1)
            attn_p = pool(name="attn", bufs=1)
            const_p = pool(name="const", bufs=1)
            exp_p = pool(name="exps", bufs=4)
            rope_p = pool(name="rope", bufs=2)

            qT = qT_p.tile([128, 16, NQ], BF16)
            kT = kT_p.tile([128, 4, S], BF16)
            v_sb = v_p.tile([128, 8, 8, 65], BF16)
            attn_sb = attn_p.tile([128, 16, NQ], BF16)

            # ---- constants ----
            ident = const_p.tile([128, 128], F32)
            nc.sync.dma_start(ident[:], ident_in[:])

            nc.vector.memset(v_sb[:, :, :, 64], 1.0)

            with ExitStack() as ictx:
                ipool = lambda *a, **k: ictx.enter_context(tc.tile_pool(*a, **k))
                hsT_p = ipool(name="hsT", bufs=1)
                hs32_p = ipool(name="hs32", bufs=2)
                wq_p = ipool(name="wqa", bufs=3)
                wqbf_p = ipool(name="wqbf", bufs=2)
                wk_p = ipool(name="wkbf", bufs=2)
                wv_p2 = ipool(name="wvbf", bufs=1)
                cs_p = ipool(name="cs", bufs=1)
                cs32_p = ipool(name="cs32", bufs=2)
                proj_ps = ipool(name="proj_ps", bufs=3, space="PSUM")
                tr_ps = ipool(name="tr_ps", bufs=2, space="PSUM")
                rot_ps = ipool(name="rot_ps", bufs=2, space="PSUM")

                # weight cast-DMAs (SWDGE queues run these in order)
                wv_bf = wv_p2.tile([128, 16, 512], BF16)
                nc.gpsimd.dma_start(wv_bf[:], wv_r[:])
                wk_bfs = []
                for p in range(4):
                    wk_bf = wk_p.tile([128, 16, 128], BF16, tag="wkbf")
                    nc.gpsimd.dma_start(wk_bf[:], wk_r[:, :, 128 * p:128 * (p + 1)])
                    wk_bfs.append(wk_bf)

                # ---- hs -> hsT (PE transpose, bf16 on the copy out) ----
                hs_ctxT = hsT_p.tile([128, 16, S], BF16)
                hs_qT = hsT_p.tile([128, 16, NQ], BF16)

                def transpose_hs(src_r, ntile, dst, dst_off):
                    for tt in range(ntile):
                        halves = []
                        for hh in range(2):
                            h32 = hs32_p.tile([128, D // 2], F32, tag="h32")
                            nc.sync.dma_start(
                                h32[:], src_r[:, tt, hh * 1024:(hh + 1) * 1024])
                            halves.append(h32)
                        for ktg in range(4):
                            pt = tr_ps.tile([128, 512], F32, tag="trp")
                            for j in range(4):
                                kt = 4 * ktg + j
                                nc.tensor.transpose(
                                    pt[:, 128 * j:128 * (j + 1)],
                                    halves[kt // 8][:, (kt % 8) * 128:(kt % 8 + 1) * 128],
                                    ident[:])
                            eng = nc.scalar if (tt + ktg) % 2 else nc.vector
                            cp = (nc.scalar.copy if (tt + ktg) % 2
                                  else nc.vector.tensor_copy)
                            cp(dst[:, 4 * ktg:4 * ktg + 4,
                                   dst_off + tt * 128:dst_off + (tt + 1) * 128],
                               pt.rearrange("p (k x) -> p k x", k=4))
                transpose_hs(hsc_r, 8, hs_ctxT, 0)
                transpose_hs(hsq_r, 4, hs_qT, 0)

                # ---- cos/sin -> [128, n] bf16 via PE transpose ----
                def load_cs(cos_d, sin_d, n, tagn):
                    c4 = cs_p.tile([128, n], BF16, tag=f"c4_{tagn}")
                    s4 = cs_p.tile([128, n], BF16, tag=f"s4_{tagn}")
                    for t, src in ((c4, cos_d), (s4, sin_d)):
                        c32 = cs32_p.tile([128, n // 128, 32], F32, tag="c32")
                        nc.sync.dma_start(
                            c32[:], src.rearrange("(o p) f -> p o f", p=128))
                        for hh in range(n // 512):
                            pt = tr_ps.tile([128, 512], F32, tag="trp", name="cospt")[0:32]
                            for o in range(4):
                                nc.tensor.transpose(
                                    pt[:, 128 * o:128 * (o + 1)],
                                    c32[:, 4 * hh + o, :], ident[:])
                            nc.scalar.copy(t[0:32, 512 * hh:512 * (hh + 1)], pt[:])
                        nc.vector.tensor_copy(t[32:64, :], t[0:32, :])
                        nc.vector.tensor_copy(t[64:128, :], t[0:64, :])
                    return c4, s4
                c4q, s4q = load_cs(cos_q, sin_q, NQ, "q")
                c4k, s4k = load_cs(cos_ctx, sin_ctx, S, "k")
                rot_bf = const_p.tile([128, 128], BF16)
                nc.sync.dma_start(rot_bf[:], rot_in[:])
                masks_bf = const_p.tile([128, 2, 4, 256], BF16)
                nc.sync.dma_start(masks_bf[:], masks_in[:])

                def rope(psum, c4, s4, col0, n, dst):
                    """psum [128, n] -> dst (bf16) with NeoX rope applied."""
                    x_sb = rope_p.tile([128, n], BF16, tag="rsb")
                    nc.scalar.copy(x_sb[:], psum[:])
                    pr = rot_ps.tile([128, n], F32, tag="rps")
                    nc.tensor.matmul(pr[:], rot_bf[:], x_sb[:], start=True, stop=True)
                    t1 = rope_p.tile([128, n], BF16, tag="rt1")
                    nc.vector.tensor_tensor(t1[:], pr[:], s4[:, col0:col0 + n], MUL)
                    t2 = rope_p.tile([128, n], BF16, tag="rt2")
                    nc.vector.tensor_tensor(t2[:], x_sb[:], c4[:, col0:col0 + n], MUL)
                    nc.vector.tensor_tensor(dst, t1[:], t2[:], ADD)

                # ---- K projection + rope ----
                for p in range(4):
                    for ch in range(2):
                        pk = proj_ps.tile([128, 512], F32, tag="proj")
                        for kt in range(16):
                            nc.tensor.matmul(
                                pk[:], wk_bfs[p][:, kt, :],
                                hs_ctxT[:, kt, 512 * ch:512 * (ch + 1)],
                                start=(kt == 0), stop=(kt == 15))
                        rope(pk, c4k, s4k, 512 * ch, 512,
                             kT[:, p, 512 * ch:512 * (ch + 1)])

                # ---- V projection (natural layout + ones column) ----
                for tt in range(8):
                    pv32 = proj_ps.tile([128, 512], F32, tag="proj")
                    for kt in range(16):
                        nc.tensor.matmul(
                            pv32[:], hs_ctxT[:, kt, tt * 128:(tt + 1) * 128],
                            wv_bf[:, kt, :], start=(kt == 0), stop=(kt == 15))
                    nc.vector.tensor_copy(
                        v_sb[:, tt, :, 0:64],
                        pv32.rearrange("p (g c) -> p g c", g=8))

                # ---- Q projection + rope ----
                for a in range(4):
                    wqa = wq_p.tile([128, 8, 512], BF16, tag="wqa")
                    wqb = wq_p.tile([128, 8, 512], BF16, tag="wqa")
                    nc.gpsimd.dma_start(wqa[:], wq_r[:, 0:8, a, :])
                    nc.gpsimd.dma_start(wqb[:], wq_r[:, 8:16, a, :])
                    for i in range(4):
                        s = 4 * a + i
                        wq_bf = wqbf_p.tile([128, 16, 128], BF16, tag="wqbf")
                        for half, w in ((0, wqa), (1, wqb)):
                            src = w.rearrange("p k (r two x) -> p k r two x",
                                              r=2, two=4)[:, :, :, i, :]
                            nc.vector.tensor_copy(
                                wq_bf[:, 8 * half:8 * (half + 1), :].rearrange(
                                    "p k (r x) -> p k r x", r=2), src)
                        pq = proj_ps.tile([128, 512], F32, tag="proj")
                        for kt in range(16):
                            nc.tensor.matmul(
                                pq[:], wq_bf[:, kt, :], hs_qT[:, kt, :],
                                start=(kt == 0), stop=(kt == 15))
                        rope(pq, c4q, s4q, 0, NQ, qT[:, s, :])

            # ---- attention (256-q-chunks, fused exp/mask) ----
            osb_p = pool(name="osb", bufs=2)
            wobf_p = pool(name="wobf", bufs=8)
            norm_p = pool(name="norm", bufs=2)
            with ExitStack() as actx:
                apool = lambda *a, **k: actx.enter_context(tc.tile_pool(*a, **k))
                sc_ps = apool(name="sc_ps", bufs=2, space="PSUM")
                pv_ps = apool(name="pv_ps", bufs=4, space="PSUM")
                for sl2 in range(2):
                    nkb = CSLOT2[sl2]
                    for g in range(8):
                        a, par = g // 2, g % 2
                        base = 64 * par
                        pvs = [pv_ps.tile([65, 4, 128], F32, tag="pv",
                                          name=f"pv{h}") for h in range(2)]
                        for kb in range(nkb):
                            sc = sc_ps.tile([128, 2, 4, 128], F32, tag="sc")
                            for h in range(2):
                                nc.tensor.matmul(
                                    sc[:, h],
                                    kT[base:base + 64, a, kb * 128:(kb + 1) * 128],
                                    qT[base:base + 64, 4 * a:4 * a + 4,
                                       (2 * sl2 + h) * 128:(2 * sl2 + h + 1) * 128],
                                    start=True, stop=True)
                            ex = exp_p.tile([128, 2, 4, 128], BF16, tag="ex")
                            nc.scalar.activation(ex[:], sc[:], AF.Exp, scale=SCALE)
                            if kb in MASK_POS2[sl2]:
                                mi = MASK_POS2[sl2].index(kb)
                                mk = masks_bf[:, sl2, mi].rearrange(
                                    "p (two x) -> p two x", two=2)[:, :, None, :]
                                nc.vector.tensor_tensor(
                                    ex[:], ex[:],
                                    mk.to_broadcast((128, 2, 4, 128)), MUL)
                            for h in range(2):
                                nc.tensor.matmul(
                                    pvs[h][:], v_sb[:, kb, g, :], ex[:, h],
                                    start=(kb == 0), stop=(kb == nkb - 1))
                        l_sb = norm_p.tile([1, 2, 4, 128], F32, tag="lsb")
                        nc.scalar.copy(l_sb[:, 0], pvs[0][64:65, :, :])
                        nc.vector.tensor_copy(l_sb[:, 1], pvs[1][64:65, :, :])
                        rc2 = norm_p.tile([1, 2, 4, 128], F32, tag="recip")
                        nc.vector.reciprocal_approx_fast(
                            rc2.rearrange("p a b q -> p (a b q)"),
                            l_sb.rearrange("p a b q -> p (a b q)"))
                        rb2 = norm_p.tile([64, 2, 4, 128], F32, tag="rb")
                        nc.gpsimd.partition_broadcast(rb2[:], rc2[:])
                        for h in range(2):
                            sl = 2 * sl2 + h
                            pv = pvs[h]
                            pv_pair = pv[0:64].rearrange(
                                "p (i two) q -> p two i q", two=2)
                            rb_pair = rb2[:, h].rearrange(
                                "p (i two) q -> p two i q", two=2)
                            for par_o in range(2):
                                nc.vector.tensor_tensor(
                                    attn_sb[64 * par_o:64 * par_o + 64,
                                            2 * g:2 * g + 2,
                                            sl * 128:(sl + 1) * 128],
                                    pv_pair[:, par_o], rb_pair[:, par_o], MUL)
            po_ps = pool(name="po_ps", bufs=3, space="PSUM")

            # ---- output projection (256-wide chunks) ----
            wo_bfs = []
            for oc in range(8):
                wo_bf = wobf_p.tile([128, 16, 256], BF16, tag="wobf")
                nc.gpsimd.dma_start(wo_bf[:], wo_r[:, :, 256 * oc:256 * (oc + 1)])
                wo_bfs.append(wo_bf)
            for oc in range(8):
                wo_bf = wo_bfs[oc]
                for tt in range(4):
                    po = po_ps.tile([128, 256], F32, tag="po")
                    for cht in range(16):
                        nc.tensor.matmul(
                            po[:], attn_sb[:, cht, tt * 128:(tt + 1) * 128],
                            wo_bf[:, cht, :], start=(cht == 0), stop=(cht == 15))
                    o_sb = osb_p.tile([128, 256], F32, tag="osb")
                    nc.scalar.copy(o_sb[:], po[:])
                    nc.sync.dma_start(
                        out[tt * 128:(tt + 1) * 128, 256 * oc:256 * (oc + 1)],
                        o_sb[:])

    nc.finalize()
    return nc


def _core_rows(c):
    p, which = c // 2, c % 2
    if which == 0:
        rel = np.r_[np.arange(256), np.arange(768, 1024)]
        ctx = 1024
    else:
        rel = np.arange(256, 768)
        ctx = 768
    return p, rel, ctx


def _host_consts():
    rot = np.zeros((128, 128), np.float32)
    for o in (0, 64):
        for d in range(32):
            rot[o + 32 + d, o + d] = -1.0
            rot[o + d, o + 32 + d] = 1.0
    tri = (np.arange(128)[None, :] >= np.arange(128)[:, None]).astype(np.float32)
    return rot.astype(ml_dtypes.bfloat16), tri, np.eye(128, dtype=np.float32)


_NC_CACHE = {}
_LAST_INMAPS = None


def kernel(hidden_states, cos, sin, Wq, Wk, Wv, Wo):
    hidden_states = np.ascontiguousarray(hidden_states, dtype=np.float32)
    cos = np.ascontiguousarray(cos, dtype=np.float32)
    sin = np.ascontiguousarray(sin, dtype=np.float32)
    Wq = np.ascontiguousarray(Wq, dtype=np.float32)
    Wk = np.ascontiguousarray(Wk, dtype=np.float32)
    Wv = np.ascontiguousarray(Wv, dtype=np.float32)
    Wo = np.ascontiguousarray(Wo, dtype=np.float32)

    if "nc" not in _NC_CACHE:
        _NC_CACHE["nc"] = build_nc()
    nc = _NC_CACHE["nc"]

    rot, tri, ident = _host_consts()
    in_maps = []
    for c in range(8):
        p, rel, ctx = _core_rows(c)
        rows = p * S + rel
        hs_ctx = np.zeros((S, D), np.float32)
        hs_ctx[:ctx] = hidden_states[p * S:p * S + ctx]
        masks = np.ones((128, 2, 4, 256), np.float32)
        for sl2 in range(2):
            qabs = rel[sl2 * 256:(sl2 + 1) * 256]
            for mi, pos in enumerate(MASK_POS2[sl2]):
                kabs = pos * 128 + np.arange(128)
                masks[:, sl2, mi, :] = (qabs[None, :] >= kabs[:, None])
        in_maps.append(dict(
            hs_ctx=hs_ctx,
            hs_q=np.ascontiguousarray(hidden_states[rows]),
            cos_ctx=np.ascontiguousarray(cos[p * S:(p + 1) * S]),
            sin_ctx=np.ascontiguousarray(sin[p * S:(p + 1) * S]),
            cos_q=np.ascontiguousarray(cos[p * S + rel]),
            sin_q=np.ascontiguousarray(sin[p * S + rel]),
            wq=Wq, wk=Wk, wv=Wv, wo=Wo,
            rot=rot, masks=masks.astype(ml_dtypes.bfloat16), ident=ident,
        ))

    global _LAST_INMAPS
    _LAST_INMAPS = in_maps

    last_err = None
    for _attempt in range(2):
        try:
            res = run_bass_kernel_spmd(nc, in_maps, core_ids=list(range(8)))
            break
        except Exception as e:  # one retry: device occasionally needs a reset
            last_err = e
    else:
        raise last_err

    outp = np.zeros((B * S, D), np.float32)
    for c in range(8):
        p, rel, ctx = _core_rows(c)
        outp[p * S + rel] = res.results[c]["out"]
    return outp
